# revision 4
# baseline (speedup 1.0000x reference)
"""Trainium2 Bass kernel for nn_MultiHeadAttention_82446192214635.

v3: interleaved emission — v-projection overlapped with first attention
block, q projections sprinkled into attention-phase PE gaps, outproj deferred
one sc to avoid head-of-line blocking on the in-order PE queue. Stage tiles
are 512-column quarters so the ring rotates quickly; the bias multiply is
done in-place on the exp tile.

Same host-side prep contract as kernel.py (bf16 pre-transposed inputs).
"""

import numpy as np

B, S, E = 4, 2048, 1024
H, DH = 16, 64
HL = 8
DL = HL * DH
N_CORES = 8

_NC_CACHE = {}


def build_nc(s=S, e=E, repeat=1):
    import concourse.bass as bass
    import concourse.tile as tile
    from concourse import bacc, mybir

    f32 = mybir.dt.float32
    bf16 = mybir.dt.bfloat16
    Exp = mybir.ActivationFunctionType.Exp

    ST = s // 128
    ES = e // 128
    SC = s // 512
    NP = HL // 2

    nc = bacc.Bacc("TRN2", target_bir_lowering=False, debug=False,
                   num_devices=N_CORES)

    xq_d = nc.dram_tensor("xq", [e, s], bf16, kind="ExternalInput")
    xk_d = nc.dram_tensor("xk", [e, s], bf16, kind="ExternalInput")
    xv_d = nc.dram_tensor("xv", [e, s], bf16, kind="ExternalInput")
    eb_d = nc.dram_tensor("eb", [s, s], bf16, kind="ExternalInput")  # bias^T
    wq_d = nc.dram_tensor("wq", [e, DL], bf16, kind="ExternalInput")
    wk_d = nc.dram_tensor("wk", [e, DL], bf16, kind="ExternalInput")
    wv_d = nc.dram_tensor("wv", [e, DL], bf16, kind="ExternalInput")
    bq_d = nc.dram_tensor("bq", [DL], f32, kind="ExternalInput")
    bk_d = nc.dram_tensor("bk", [DL], f32, kind="ExternalInput")
    bv_d = nc.dram_tensor("bv", [DL], f32, kind="ExternalInput")
    wo_d = nc.dram_tensor("wo", [DL, e], bf16, kind="ExternalInput")
    out_d = nc.dram_tensor("out", [s, e], f32, kind="ExternalOutput")

    def one_pass(tc, outbuf):
        with (
            tc.tile_pool(name="consts", bufs=1) as consts,
            tc.tile_pool(name="persist", bufs=1) as persist,
            tc.tile_pool(name="stage", bufs=2) as stage,
            tc.tile_pool(name="ebp", bufs=2) as ebp,
            tc.tile_pool(name="pps", bufs=2, space="PSUM") as pps,
            tc.tile_pool(name="sc_ps", bufs=2, space="PSUM") as sc_ps,
            tc.tile_pool(name="pv_ps", bufs=2, space="PSUM") as pv_ps,
            tc.tile_pool(name="work", bufs=3) as work,
            tc.tile_pool(name="ctxw", bufs=2) as ctxw,
            tc.tile_pool(name="sumsw", bufs=2) as sumsw,
        ):
            wq_sb = consts.tile([128, ES, DL], bf16, tag="wq")
            wk_sb = consts.tile([128, ES, DL], bf16, tag="wk")
            wv_sb = consts.tile([128, ES, DL], bf16, tag="wv")
            wo_sb = consts.tile([128, NP, e], bf16, tag="wo")
            bqk_sb = consts.tile([128, 2 * NP], f32, tag="bqk")
            bv_row = consts.tile([1, DL], f32, tag="bv_row")
            bv_bc = consts.tile([128, DL], f32, tag="bv_bc")

            qT2 = persist.tile([128, NP, s], bf16, tag="qT2")
            kT2 = persist.tile([128, NP, s], bf16, tag="kT2")
            v_sb = persist.tile([128, ST, HL * 65], bf16, tag="v_sb")
            ctxT2 = persist.tile([128, NP, s], bf16, tag="ctxT2")

            def load_w(dst, src):
                nc.sync.dma_start(
                    out=dst[:],
                    in_=src.ap().rearrange("(a p) d -> p a d", p=128))

            def load_x(src, c):
                xT = stage.tile([128, ES, 512], bf16, tag="xT")
                nc.sync.dma_start(
                    out=xT[:],
                    in_=src.ap().rearrange("(es p) s -> p es s", p=128)
                    [:, :, c * 512:(c + 1) * 512])
                return xT

            def load_eb(c):
                # one 512-column chunk of bias^T; exp'd in place on ScalarE
                # (idle during staging). Chunk c serves attn sc=c.
                ebc = ebp.tile([128, ST, 512], bf16, tag="ebc")
                nc.sync.dma_start(
                    out=ebc[:],
                    in_=eb_d.ap().rearrange("(ts p) s -> p ts s", p=128)
                    [:, :, c * 512:(c + 1) * 512])
                nc.scalar.activation(out=ebc[:], in_=ebc[:], func=Exp)
                return ebc

            def qk_chain(dst, w_sb, bcol, xT, c, p):
                # projection of s-chunk c (columns c*512..) for pair p
                ps = pps.tile([128, 512], f32, tag="pps")
                for es in range(ES):
                    nc.tensor.matmul(
                        ps[:],
                        lhsT=w_sb[:, es, p * 128:(p + 1) * 128],
                        rhs=xT[:, es, :],
                        start=(es == 0), stop=(es == ES - 1))
                nc.vector.tensor_scalar_add(
                    out=dst[:, p, c * 512:(c + 1) * 512],
                    in0=ps[:], scalar1=bqk_sb[:, bcol + p:bcol + p + 1])

            def v_chain(xT, tt):
                tl = tt % 4
                ps = pps.tile([128, 512], f32, tag="pps")
                for es in range(ES):
                    nc.tensor.matmul(
                        ps[:],
                        lhsT=xT[:, es, tl * 128:(tl + 1) * 128],
                        rhs=wv_sb[:, es, :],
                        start=(es == 0), stop=(es == ES - 1))
                nc.vector.tensor_add(
                    out=v_sb[:, tt, :].rearrange(
                        "p (h c) -> p h c", h=HL)[:, :, 0:64],
                    in0=ps[:].rearrange("p (h d) -> p h d", h=HL),
                    in1=bv_bc[:].rearrange("p (h d) -> p h d", h=HL))

            def attn_iter(sc, p, tt, pv0, pv1, ebc):
                s0 = sc * 512
                scp = sc_ps.tile([128, 1024], f32, tag="scp")
                for hh in range(2):
                    nc.tensor.matmul(
                        scp[:, hh * 512:(hh + 1) * 512],
                        lhsT=kT2[hh * 64:(hh + 1) * 64, p,
                                 tt * 128:(tt + 1) * 128],
                        rhs=qT2[hh * 64:(hh + 1) * 64, p, s0:s0 + 512],
                        start=True, stop=True)
                expt = work.tile([128, 1024], bf16, tag="expt")
                nc.scalar.activation(
                    out=expt[:], in_=scp[:], func=Exp, scale=0.125)
                eb = ebc[:, tt, :]
                eb_rep = bass.AP(
                    tensor=eb.tensor, offset=eb.offset,
                    ap=[list(eb.ap[0]), [0, 2], [1, 512]])
                mul_eng = nc.gpsimd if tt % 2 == 0 else nc.vector
                mul_eng.tensor_mul(out=expt[:], in0=expt[:], in1=eb_rep)
                for hh, pv in ((0, pv0), (1, pv1)):
                    h = 2 * p + hh
                    nc.tensor.matmul(
                        pv[:],
                        lhsT=v_sb[:, tt, h * 65:(h + 1) * 65],
                        rhs=expt[:, hh * 512:(hh + 1) * 512],
                        start=(tt == 0), stop=(tt == ST - 1))

            def attn_drain(sc, p, pv0, pv1, sums_p):
                s0 = sc * 512
                ctxun = ctxw.tile([128, 512], bf16, tag="ctxun")
                nc.vector.tensor_copy(out=ctxun[0:64, :], in_=pv0[0:64, :])
                nc.vector.tensor_copy(out=ctxun[64:128, :], in_=pv1[0:64, :])
                nc.vector.tensor_copy(out=sums_p[0:1, 0:512], in_=pv0[64:65, :])
                nc.vector.tensor_copy(out=sums_p[0:1, 512:1024],
                                      in_=pv1[64:65, :])
                recip_p = sumsw.tile([1, 1024], f32, tag="recip")
                nc.vector.reciprocal_approx_fast(out=recip_p[:], in_=sums_p[:])
                recipb = ctxw.tile([128, 512], f32, tag="recipb")
                nc.gpsimd.partition_broadcast(
                    out_ap=recipb[0:64, :], in_ap=recip_p[0:1, 0:512])
                rb1 = ctxw.tile([64, 512], f32, tag="rb1")
                nc.gpsimd.partition_broadcast(
                    out_ap=rb1[:], in_ap=recip_p[0:1, 512:1024])
                nc.vector.tensor_copy(out=recipb[64:128, :], in_=rb1[:])
                nc.vector.tensor_mul(
                    out=ctxT2[:, p, s0:s0 + 512], in0=ctxun[:], in1=recipb[:])

            def attn_block(sc, p, ebc):
                sums_p = sumsw.tile([1, 1024], f32, tag="sums")
                pv0 = pv_ps.tile([65, 512], f32, tag="pv")
                pv1 = pv_ps.tile([65, 512], f32, tag="pv")
                for tt in range(ST):
                    attn_iter(sc, p, tt, pv0, pv1, ebc)
                attn_drain(sc, p, pv0, pv1, sums_p)

            def outproj(sc):
                for m in range(4):
                    sm = sc * 4 + m
                    for eh in range(e // 512):
                        po = pps.tile([128, 512], f32, tag="pps")
                        for p in range(NP):
                            nc.tensor.matmul(
                                po[:],
                                lhsT=ctxT2[:, p, sm * 128:(sm + 1) * 128],
                                rhs=wo_sb[:, p, eh * 512:(eh + 1) * 512],
                                start=(p == 0), stop=(p == NP - 1))
                        ob = outbuf.tile([128, 512], f32, tag="ob")
                        nc.vector.tensor_copy(out=ob[:], in_=po[:])
                        nc.sync.dma_start(
                            out=out_d.ap()[sm * 128:(sm + 1) * 128,
                                           eh * 512:(eh + 1) * 512],
                            in_=ob[:])

            # ------------- emission schedule -------------
            load_w(wk_sb, wk_d)
            xk = [load_x(xk_d, c) for c in range(4)]
            load_w(wq_sb, wq_d)
            nc.sync.dma_start(
                out=bqk_sb[:, 0:NP],
                in_=bq_d.ap().rearrange("(np p) -> p np", p=128))
            nc.sync.dma_start(
                out=bqk_sb[:, NP:2 * NP],
                in_=bk_d.ap().rearrange("(np p) -> p np", p=128))
            xq0 = load_x(xq_d, 0)
            eb0 = load_eb(0)
            load_w(wv_sb, wv_d)
            nc.sync.dma_start(
                out=bv_row[:], in_=bv_d.ap().rearrange("(o d) -> o d", o=1))
            nc.gpsimd.partition_broadcast(out_ap=bv_bc[:], in_ap=bv_row[:])
            nc.vector.memset(
                v_sb[:].rearrange("p t (h c) -> p t h c", h=HL)
                [:, :, :, 64:65], 1.0)
            xv = [load_x(xv_d, c) for c in range(4)]
            load_w(wo_sb, wo_d)

            # k projections, c-outer so the stage ring rotates fast
            for c in range(4):
                for p in range(NP):
                    qk_chain(kT2, wk_sb, NP, xk[c], c, p)
            # q projections for sc0
            for p in range(NP):
                qk_chain(qT2, wq_sb, 0, xq0, 0, p)

            # v projections interleaved with attention block (0,0)
            sums_p = sumsw.tile([1, 1024], f32, tag="sums")
            pv0 = pv_ps.tile([65, 512], f32, tag="pv")
            pv1 = pv_ps.tile([65, 512], f32, tag="pv")
            for tt in range(ST):
                v_chain(xv[tt // 4], tt)
                attn_iter(0, 0, tt, pv0, pv1, eb0)
            attn_drain(0, 0, pv0, pv1, sums_p)

            attn_block(0, 1, eb0)
            xq1 = load_x(xq_d, 1)
            for p in range(NP):
                qk_chain(qT2, wq_sb, 0, xq1, 1, p)
            eb1 = load_eb(1)
            attn_block(0, 2, eb0)
            xq2 = load_x(xq_d, 2)
            for p in range(NP):
                qk_chain(qT2, wq_sb, 0, xq2, 2, p)
            attn_block(0, 3, eb0)

            attn_block(1, 0, eb1)
            outproj(0)
            attn_block(1, 1, eb1)
            xq3 = load_x(xq_d, 3)
            for p in range(NP):
                qk_chain(qT2, wq_sb, 0, xq3, 3, p)
            eb2 = load_eb(2)
            attn_block(1, 2, eb1)
            attn_block(1, 3, eb1)

            attn_block(2, 0, eb2)
            outproj(1)
            attn_block(2, 1, eb2)
            eb3 = load_eb(3)
            attn_block(2, 2, eb2)
            attn_block(2, 3, eb2)

            attn_block(3, 0, eb3)
            outproj(2)
            attn_block(3, 1, eb3)
            attn_block(3, 2, eb3)
            attn_block(3, 3, eb3)
            outproj(3)

    with tile.TileContext(nc) as tc:
        with tc.tile_pool(name="outbuf", bufs=2) as outbuf:
            for _rep in range(repeat):
                one_pass(tc, outbuf)

    nc.compile()
    return nc


def shard_inputs(inputs):
    """Full fp32 inputs -> per-core in_maps. Host does layout-only prep:
    transposes, dtype casts, sharding. All FLOPs stay on device."""
    import ml_dtypes
    bf16 = ml_dtypes.bfloat16

    ins = {k: np.asarray(v, dtype=np.float32) for k, v in inputs.items()}
    xT = {}
    for nm, key in (("xq", "query"), ("xk", "key"), ("xv", "value")):
        xT[nm] = [np.ascontiguousarray(ins[key][b].T).astype(bf16)
                  for b in range(B)]
    ebT = np.ascontiguousarray(ins["attention_bias"].T).astype(bf16)

    in_maps = []
    for c in range(N_CORES):
        b, g = c // 2, c % 2
        hs = slice(g * HL, (g + 1) * HL)
        in_maps.append({
            "xq": xT["xq"][b],
            "xk": xT["xk"][b],
            "xv": xT["xv"][b],
            "eb": ebT,
            "wq": np.ascontiguousarray(
                ins["Wq"][hs].transpose(1, 0, 2).reshape(E, DL)).astype(bf16),
            "wk": np.ascontiguousarray(
                ins["Wk"][hs].transpose(1, 0, 2).reshape(E, DL)).astype(bf16),
            "wv": np.ascontiguousarray(
                ins["Wv"][hs].transpose(1, 0, 2).reshape(E, DL)).astype(bf16),
            "bq": np.ascontiguousarray(ins["bq"][hs].reshape(DL)),
            "bk": np.ascontiguousarray(ins["bk"][hs].reshape(DL)),
            "bv": np.ascontiguousarray(ins["bv"][hs].reshape(DL)),
            "wo": np.ascontiguousarray(
                ins["Wo"][g * DL:(g + 1) * DL]).astype(bf16),
        })
    return in_maps


def kernel(**inputs):
    from concourse.bass_utils import run_bass_kernel_spmd

    nc = _NC_CACHE.get("nc")
    if nc is None:
        nc = _NC_CACHE["nc"] = build_nc()

    in_maps = shard_inputs(inputs)
    res = run_bass_kernel_spmd(nc, in_maps, core_ids=list(range(N_CORES)))
    parts = [r["out"] for r in res.results]

    bo = np.asarray(inputs["bo"], dtype=np.float32)
    out = np.empty((B, S, E), np.float32)
    for b in range(B):
        out[b] = parts[2 * b] + parts[2 * b + 1] + bo[None, :]
    return out
